# revision 1
# baseline (speedup 1.0000x reference)
"""Trainium2 Bass kernel for local-correlation + masked top-256 (sparse_attention).

Contract: kernel(**inputs) takes FULL unsharded inputs (pre, curr, mask, mode)
and returns the full output tuple (f, b), each [4, 256, 128, 128] f32.

Sharding: pure data parallel over (batch, H-half) -> 8 cores.
Per core:
  - L2-normalize pre/curr over C (sumsq via ones-matmul, invn = Exp(-0.5*Ln(ss))).
  - For each of 64 output rows h: 17 Gram matmuls cur_n[:,h,:]^T @ pre_n[:,h+dy,:]
    -> [128w, 144w'] in PSUM; copied into a staging tile, round-tripped through
    DRAM with write-row-stride 2449 and read-partition-stride 2450 so the
    diagonal band co[w, dy*17+dx] = g[dy][w, w+dx] comes back as one DMA.
  - xf = co*m (mask unfolded on host), xb = co - xf.
  - top-256 sorted desc via 32 rounds of vector max8 + match_replace(-3.0).
  - PE transpose [w,k] -> [k,w], DMA to [256, 64, 128] outputs.
"""

import numpy as np

K = 8
KW = 17
D = KW * KW            # 289
TOPK = 256
B, C, H, W = 4, 256, 128, 128
N_CORES = 8
HSLICE = H // 2        # 64 rows per core
WP = W + 2 * K         # 144
HP = HSLICE + 2 * K    # 80
NROUND = TOPK // 8     # 32
PRE_COLS = HP * WP     # 11520
CUR_COLS = HSLICE * W  # 8192
STG = KW * WP          # 2448
SCR_FLAT = 128 * (STG + 2)   # 313600 : divisible by 2450; first 128*2449 used for write view

_CACHED = {"nc": None}


def _build_nc():
    import concourse.bacc as bacc
    import concourse.tile as tile
    import concourse.mybir as mybir

    f32 = mybir.dt.float32
    AF = mybir.ActivationFunctionType
    ALU = mybir.AluOpType

    nc = bacc.Bacc("TRN2", target_bir_lowering=False, debug=False,
                   enable_asserts=False, num_devices=N_CORES)

    pre_d = nc.dram_tensor("pre_pad", [C, HP, WP], f32, kind="ExternalInput").ap()
    cur_d = nc.dram_tensor("curr", [C, HSLICE, W], f32, kind="ExternalInput").ap()
    m_d = nc.dram_tensor("m_unf", [HSLICE, W, D], f32, kind="ExternalInput").ap()
    f_d = nc.dram_tensor("f_out", [TOPK, HSLICE, W], f32, kind="ExternalOutput").ap()
    b_d = nc.dram_tensor("b_out", [TOPK, HSLICE, W], f32, kind="ExternalOutput").ap()
    scr = [nc.dram_tensor(f"scr{i}", [SCR_FLAT], f32, kind="Internal").ap()
           for i in range(2)]

    ident_d = nc.inline_tensor(np.eye(128, dtype=np.float32), name="ident")
    ones_col_d = nc.inline_tensor(np.ones((128, 1), np.float32), name="ones_col")
    ones_row_d = nc.inline_tensor(np.ones((1, 128), np.float32), name="ones_row")

    with tile.TileContext(nc) as tc:
        with tc.tile_pool(name="persist", bufs=1) as pp:
            pre_n = [pp.tile([128, PRE_COLS], f32, tag=f"pre{c}", name=f"pre{c}") for c in range(2)]
            cur_n = [pp.tile([128, CUR_COLS], f32, tag=f"cur{c}", name=f"cur{c}") for c in range(2)]
            ident = pp.tile([128, 128], f32, tag="ident", name="identt")
            ones_col = pp.tile([128, 1], f32, tag="onc", name="onc")
            ones_row = pp.tile([1, 128], f32, tag="onr", name="onr")
            nc.sync.dma_start(ident[:, :], ident_d.ap())
            nc.sync.dma_start(ones_col[:, :], ones_col_d.ap())
            nc.sync.dma_start(ones_row[:, :], ones_row_d.ap())
            for c in range(2):
                nc.sync.dma_start(
                    pre_n[c][:, :],
                    pre_d[c * 128:(c + 1) * 128, :, :].rearrange("c hh ww -> c (hh ww)"))
                nc.sync.dma_start(
                    cur_n[c][:, :],
                    cur_d[c * 128:(c + 1) * 128, :, :].rearrange("c hh ww -> c (hh ww)"))

            # ---- Stage A: L2 normalization over C (in place) ----
            with (tc.tile_pool(name="sqp", bufs=2) as sqp,
                  tc.tile_pool(name="strip", bufs=2) as stp,
                  tc.tile_pool(name="ssp", bufs=2, space="PSUM") as ssp,
                  tc.tile_pool(name="bcp", bufs=2, space="PSUM") as bcp):
                for tiles, ncols, cw in ((pre_n, PRE_COLS, 480), (cur_n, CUR_COLS, 512)):
                    for j in range(ncols // cw):
                        cs = slice(j * cw, (j + 1) * cw)
                        ss = ssp.tile([1, cw], f32, tag="ss", name="ss")
                        for c in range(2):
                            sq = sqp.tile([128, cw], f32, tag="sq", name="sq")
                            nc.scalar.activation(sq[:, :], tiles[c][:, cs], AF.Square)
                            nc.tensor.matmul(ss[:, :], ones_col[:, :], sq[:, :],
                                             start=(c == 0), stop=(c == 1))
                        lns = stp.tile([1, cw], f32, tag="lns", name="lns")
                        nc.scalar.activation(lns[:, :], ss[:, :], AF.Ln)
                        bc = bcp.tile([128, cw], f32, tag="bc", name="bc")
                        nc.tensor.matmul(bc[:, :], ones_row[:, :], lns[:, :],
                                         start=True, stop=True)
                        inv = sqp.tile([128, cw], f32, tag="inv", name="inv")
                        nc.scalar.activation(inv[:, :], bc[:, :], AF.Exp, scale=-0.5)
                        for c in range(2):
                            nc.gpsimd.tensor_tensor(
                                out=tiles[c][:, cs], in0=tiles[c][:, cs],
                                in1=inv[:, :], op=ALU.mult)

            # ---- Stage B: per-row gram, shear, mask, top-k extraction ----
            with (tc.tile_pool(name="stage", bufs=2) as stgp,
                  tc.tile_pool(name="cop", bufs=2) as cop,
                  tc.tile_pool(name="xp", bufs=4) as xp,
                  tc.tile_pool(name="fbp", bufs=4) as fbp,
                  tc.tile_pool(name="trp", bufs=2) as trp,
                  tc.tile_pool(name="gp", bufs=4, space="PSUM") as gp,
                  tc.tile_pool(name="tp", bufs=4, space="PSUM") as tp):
                for h in range(HSLICE):
                    stage = stgp.tile([128, STG], f32, tag="stage", name="stage")
                    for dy in range(KW):
                        g = gp.tile([128, WP], f32, tag="g", name="g")
                        for c in range(2):
                            nc.tensor.matmul(
                                g[:, :],
                                cur_n[c][:, h * W:(h + 1) * W],
                                pre_n[c][:, (h + dy) * WP:(h + dy + 1) * WP],
                                start=(c == 0), stop=(c == 1))
                        nc.scalar.activation(stage[:, dy * WP:(dy + 1) * WP],
                                             g[:, :], AF.Copy)
                    sc = scr[h % 2]
                    wview = sc[0:128 * (STG + 1)].rearrange("(p r) -> p r", r=STG + 1)
                    nc.sync.dma_start(wview[:, 0:STG], stage[:, :])
                    co = cop.tile([128, D], f32, tag="co", name="co")
                    rview = sc[:].rearrange("(p r) -> p r", r=STG + 2)
                    rview = rview[:, 0:STG].rearrange("p (a b) -> p a b", b=WP)
                    nc.sync.dma_start(co[:, :], rview[:, :, 0:KW])
                    m = cop.tile([128, D], f32, tag="m", name="m")
                    nc.sync.dma_start(m[:, :], m_d[h, :, :])
                    xf = xp.tile([128, D], f32, tag="xf", name="xf")
                    xb = xp.tile([128, D], f32, tag="xb", name="xb")
                    nc.gpsimd.tensor_tensor(out=xf[:, :], in0=co[:, :], in1=m[:, :],
                                            op=ALU.mult)
                    nc.gpsimd.tensor_tensor(out=xb[:, :], in0=co[:, :], in1=xf[:, :],
                                            op=ALU.subtract)
                    for x, out_d in ((xf, f_d), (xb, b_d)):
                        ft = fbp.tile([128, TOPK], f32, tag="ft", name="ft")
                        for r in range(NROUND):
                            nc.vector.max(ft[:, r * 8:(r + 1) * 8], x[:, :])
                            if r + 1 < NROUND:
                                nc.vector.match_replace(
                                    x[:, :], ft[:, r * 8:(r + 1) * 8], x[:, :],
                                    imm_value=-3.0)
                        tr = trp.tile([128, TOPK], f32, tag="tr", name="tr")
                        for half in range(2):
                            pt = tp.tile([128, 128], f32, tag="pt", name="pt")
                            nc.tensor.transpose(
                                pt[:, :], ft[:, half * 128:(half + 1) * 128],
                                ident[:, :])
                            nc.scalar.activation(
                                tr[:, half * 128:(half + 1) * 128], pt[:, :], AF.Copy)
                        oview = out_d.rearrange("(cc p) hh ww -> p cc hh ww", cc=2)
                        nc.sync.dma_start(
                            oview[:, :, h, :],
                            tr[:, :].rearrange("p (cc ww) -> p cc ww", ww=W))
    nc.compile()
    return nc


def _host_prep(pre, curr, mask):
    pre_pad = np.pad(pre, ((0, 0), (0, 0), (K, K), (K, K)), mode="reflect")
    mask_pad = np.pad(mask, ((0, 0), (0, 0), (K, K), (K, K)))
    # unfold mask: m[h, w, dy*17+dx] = mask_pad[h+dy, w+dx]
    ins = []
    for k in range(N_CORES):
        b, hh = k // 2, k % 2
        h0 = hh * HSLICE
        mp = mask_pad[b, 0, h0:h0 + HP, :]
        s0, s1 = mp.strides
        m_unf = np.lib.stride_tricks.as_strided(
            mp, (HSLICE, KW, W, KW), (s0, s0, s1, s1))
        m_unf = np.ascontiguousarray(
            m_unf.transpose(0, 2, 1, 3).reshape(HSLICE, W, D))
        ins.append({
            "pre_pad": np.ascontiguousarray(pre_pad[b, :, h0:h0 + HP, :]),
            "curr": np.ascontiguousarray(curr[b, :, h0:h0 + HSLICE, :]),
            "m_unf": m_unf,
        })
    return ins


def kernel(pre, curr, mask, mode):
    from concourse.bass_utils import run_bass_kernel_spmd

    pre = np.asarray(pre, dtype=np.float32)
    curr = np.asarray(curr, dtype=np.float32)
    mask = np.asarray(mask, dtype=np.float32)
    assert int(np.asarray(mode)) == 0

    if _CACHED["nc"] is None:
        _CACHED["nc"] = _build_nc()
    nc = _CACHED["nc"]

    in_maps = _host_prep(pre, curr, mask)
    res = run_bass_kernel_spmd(nc, in_maps, core_ids=list(range(N_CORES)))
    f = np.zeros((B, TOPK, H, W), np.float32)
    bo = np.zeros((B, TOPK, H, W), np.float32)
    for k in range(N_CORES):
        bb, hh = k // 2, k % 2
        f[bb, :, hh * HSLICE:(hh + 1) * HSLICE, :] = res.results[k]["f_out"]
        bo[bb, :, hh * HSLICE:(hh + 1) * HSLICE, :] = res.results[k]["b_out"]
    return (f, bo)



# revision 35
# speedup vs baseline: 2.1012x; 2.1012x over previous
"""Trainium2 Bass kernel for local-correlation + masked top-256 (sparse_attention).

Contract: kernel(**inputs) takes FULL unsharded inputs (pre, curr, mask, mode)
and returns the full output tuple (f, b), each [4, 256, 128, 128] f32.

Sharding: pure data parallel over (batch, H-half) -> 8 cores.

Per core:
  - L2-normalize pre/curr over C in fp32 (chunked: sumsq via ones-matmul,
    reciprocal on DVE, sqrt on Act -- keeps every Act func in one table),
    write bf16 row-group tiles; chunks are emitted interleaved with the row
    loop so early rows start immediately.
  - Per output row h: 17 bf16 Gram matmuls cur^T @ pre -> PSUM; bf16 staging
    tile round-trips through DRAM with a +1-element partition shear so the
    diagonal band co[w, dy*17+dx] comes back as one DMA ([128, 289] bf16).
  - y = co + 4*m (mask lifts m=1 values into [3,5], m=0 stay in [-1,1]).
    ONE full descending sort of y per pixel (37x vector.max8 + 36x
    match_replace) gives both mask-subsets as contiguous sorted segments
    split at the per-pixel mask count kappa, which is HOST-known.
  - Reconstruct both outputs from the shared sort (q = 289-kappa):
      f[r] = max(cs4[r] - 6*[r>=kappa], min(scatter(cs4, j->j+q | j<kappa), 0))
      b[r] = max(scatter(cs, j->j-kappa | j>=kappa) - 10*[r>=q], min(cs[r], 0))
    with cs = sorted y, cs4 = cs - 4; scatters are gpsimd local_scatter with
    host-built per-partition int16 indices (zero-fill, -1 skips); the maxes
    run as a + relu(b - a) on Pool+Act so the DVE does nothing but sort.
  - PE transpose [w,256] -> [256,w] halves, copy to fp32, DMA out.
"""

import numpy as np

K = 8
KW = 17
D = KW * KW            # 289
TOPK = 256
B, C, H, W = 4, 256, 128, 128
N_CORES = 8
HSLICE = H // 2        # 64 rows per core
WP = W + 2 * K         # 144
HP = HSLICE + 2 * K    # 80
NROUND = (D + 7) // 8  # 37 max8 rounds for the full sort
CS_COLS = NROUND * 8   # 296
STG = KW * WP          # 2448
SCR_FLAT = 128 * (STG + 2)   # write view uses row pitch STG+1, read STG+2
NIDXB = D + 1          # 290 (even, covers ranks 0..288 + one dead slot)
M4W = D + 1            # m4 padded to even length (col 289 = 0)
AUXB = M4W + 2 * TOPK  # bf16 aux row: m4 | hm6 | hm10
AUXI = TOPK + NIDXB    # int16 aux row: idxf | idxb

NR_EARLY = 18          # sort rounds before compaction (144 extracted)
LIVE = D - 8 * NR_EARLY          # 145 survivors
ZC_W = LIVE + 1                  # zc tile width (even)
IW = M4W               # 290, width of compaction intermediates

PRE_RPT = 4            # padded-pre rows per tile (each gram rhs row-aligned)
PRE_NT = HP // PRE_RPT          # 20 tiles of [128, 576]
PRE_TCOLS = PRE_RPT * WP        # 576
CUR_RPT = 4            # cur rows per tile
CUR_NT = HSLICE // CUR_RPT      # 16 tiles of [128, 512]
CUR_TCOLS = CUR_RPT * W         # 512

_CACHED = {"nc": None}


def _build_nc():
    import concourse.bacc as bacc
    import concourse.tile as tile
    import concourse.mybir as mybir

    f32 = mybir.dt.float32
    bf16 = mybir.dt.bfloat16
    i16 = mybir.dt.int16
    AF = mybir.ActivationFunctionType
    ALU = mybir.AluOpType

    nc = bacc.Bacc("TRN2", target_bir_lowering=False, debug=False,
                   enable_asserts=False, num_devices=N_CORES)

    pre_d = nc.dram_tensor("pre_pad", [C, HP, WP], f32, kind="ExternalInput").ap()
    cur_d = nc.dram_tensor("curr", [C, HSLICE, W], f32, kind="ExternalInput").ap()
    auxb_d = nc.dram_tensor("auxb", [HSLICE, W, AUXB], bf16, kind="ExternalInput").ap()
    auxi_d = nc.dram_tensor("auxi", [HSLICE, W, AUXI], i16, kind="ExternalInput").ap()
    f_d = nc.dram_tensor("f_out", [TOPK, HSLICE, W], f32, kind="ExternalOutput").ap()
    b_d = nc.dram_tensor("b_out", [TOPK, HSLICE, W], f32, kind="ExternalOutput").ap()
    scr = [nc.dram_tensor(f"scr{i}", [SCR_FLAT], bf16, kind="Internal").ap()
           for i in range(6)]

    iota_np = (np.arange(M4W, dtype=np.float32) - 10001.0)[None, :].repeat(128, 0)
    iota_d = nc.inline_tensor(iota_np, name="iota1")
    ident_d = nc.inline_tensor(np.eye(128, dtype=np.float32), name="ident")
    ones_col_d = nc.inline_tensor(np.ones((128, 1), np.float32), name="ones_col")
    ones_row_d = nc.inline_tensor(np.ones((1, 128), np.float32), name="ones_row")

    from contextlib import ExitStack
    with tile.TileContext(nc) as tc, ExitStack() as _stk:
        _pools = {}
        for _nm, _bufs, _spc in (
                ("pp", 1, None), ("ldp", 3, None), ("sqp", 2, None),
                ("invp", 1, None), ("stp", 2, None), ("ssq", 3, None), ("ssp", 2, "PSUM"),
                ("bcp", 1, "PSUM"), ("stgp", 3, None), ("cop", 8, None),
                ("yp", 6, None), ("csp", 8, None), ("rcp", 3, None),
                ("cpp", 2, None), ("zcp", 4, None), ("fbp", 2, None),
                ("trp", 3, None), ("gp", 3, "PSUM"), ("tp", 2, "PSUM")):
            kw = {"space": _spc} if _spc else {}
            _pools[_nm] = _stk.enter_context(
                tc.tile_pool(name=_nm, bufs=_bufs, **kw))
        pp, ldp, sqp, invp, stp, ssq, ssp, bcp, stgp, cop, yp, csp, rcp, \
            cpp, zcp, fbp, trp, gp, tp = (
                _pools[n] for n in ("pp", "ldp", "sqp", "invp", "stp", "ssq",
                                    "ssp", "bcp", "stgp", "cop", "yp", "csp",
                                    "rcp", "cpp", "zcp", "fbp", "trp", "gp",
                                    "tp"))
        if True:
            pre_nb = [[pp.tile([128, PRE_TCOLS], bf16, tag=f"pre{c}_{i}",
                               name=f"pre{c}_{i}") for i in range(PRE_NT)]
                      for c in range(2)]
            cur_nb = [[pp.tile([128, CUR_TCOLS], bf16, tag=f"cur{c}_{i}",
                               name=f"cur{c}_{i}") for i in range(CUR_NT)]
                      for c in range(2)]
            ident = pp.tile([128, 128], bf16, tag="ident", name="identt")
            identf = pp.tile([128, 128], f32, tag="identf", name="identf")
            ones_col = pp.tile([128, 1], f32, tag="onc", name="onc")
            ones_row = pp.tile([1, 128], f32, tag="onr", name="onr")
            iota1 = pp.tile([128, M4W], f32, tag="iota1", name="iota1")
            bias85 = pp.tile([128, 1], f32, tag="b85", name="b85")
            nc.sync.dma_start(iota1[:, :], iota_d.ap())
            nc.gpsimd.memset(bias85[:, :], 8.5)
            nc.sync.dma_start(identf[:, :], ident_d.ap())
            nc.scalar.activation(ident[:, :], identf[:, :], AF.Copy)
            nc.sync.dma_start(ones_col[:, :], ones_col_d.ap())
            nc.sync.dma_start(ones_row[:, :], ones_row_d.ap())

            pre_flat = pre_d.rearrange("c hh ww -> c (hh ww)")
            cur_flat = cur_d.rearrange("c hh ww -> c (hh ww)")

            def norm_p1(flat_d, i, cw):
                """phase 1: load fp32 chunk, sumsq over C (squares on Pool,
                reduce on PE), 1/sumsq on DVE straight from PSUM so the DVE
                op never waits on the congested Act stream."""
                cs_ = slice(i * cw, (i + 1) * cw)
                chunks = []
                for c in range(2):
                    ch = ldp.tile([128, 576], f32, tag=f"ch{c}", name=f"ch{c}")
                    nc.sync.dma_start(ch[:, 0:cw],
                                      flat_d[c * 128:(c + 1) * 128, cs_])
                    chunks.append(ch)
                nsub = (cw + 511) // 512
                sw = cw // nsub
                rcs = ssq.tile([1, 576], f32, tag="rcs", name="rcs")
                sss = []
                for s in range(nsub):
                    sub = slice(s * sw, (s + 1) * sw)
                    ss = ssp.tile([1, 512], f32, tag="ss", name="ss")
                    for c in range(2):
                        sq = sqp.tile([128, 512], f32, tag="sq", name="sq")
                        nc.gpsimd.tensor_tensor(out=sq[:, 0:sw],
                                                in0=chunks[c][:, sub],
                                                in1=chunks[c][:, sub],
                                                op=ALU.mult)
                        nc.tensor.matmul(ss[:, 0:sw], ones_col[:, :],
                                         sq[:, 0:sw],
                                         start=(c == 0), stop=(c == 1))
                    sss.append(ss)
                return chunks, rcs, nsub, sw, sss

            def norm_recips(state):
                """the DVE reciprocals, emitted mid-weave so the Pool/PE
                sumsq chain has already drained by the time DVE gets here"""
                chunks, rcs, nsub, sw, sss = state
                for s in range(nsub):
                    nc.vector.reciprocal(rcs[:, s * sw:(s + 1) * sw],
                                         sss[s][:, 0:sw])

            def norm_p2(dst_tiles, i, state):
                """phase 2 (two blocks later): sqrt, broadcast, multiply."""
                chunks, rcs, nsub, sw, _ = state
                for s in range(nsub):
                    sub = slice(s * sw, (s + 1) * sw)
                    srt = stp.tile([1, 512], f32, tag="srt", name="srt")
                    nc.scalar.activation(srt[:, 0:sw], rcs[:, sub], AF.Sqrt)
                    bc = bcp.tile([128, 512], f32, tag="bc", name="bc")
                    nc.tensor.matmul(bc[:, 0:sw], ones_row[:, :], srt[:, 0:sw],
                                     start=True, stop=True)
                    inv = invp.tile([128, 512], f32, tag="inv", name="inv")
                    nc.scalar.activation(inv[:, 0:sw], bc[:, 0:sw], AF.Copy)
                    for c in range(2):
                        nc.gpsimd.tensor_tensor(
                            out=dst_tiles[c][i][:, sub],
                            in0=chunks[c][:, sub],
                            in1=inv[:, 0:sw], op=ALU.mult)

            _p2q = []

            def do_chunk_p1(w, tblk):
                flat = pre_flat if w[0] == "pre" else cur_flat
                cw = PRE_TCOLS if w[0] == "pre" else CUR_TCOLS
                _p2q.append((w, norm_p1(flat, w[1], cw), tblk))

            def flush_p2(tnow):
                while _p2q and _p2q[0][2] <= tnow - 2:
                    w, state, _ = _p2q.pop(0)
                    tiles = pre_nb if w[0] == "pre" else cur_nb
                    norm_p2(tiles, w[1], state)

            def do_chunk(w):
                do_chunk_p1(w, -10)
                norm_recips(_p2q[-1][1])
                flush_p2(0)

            # prologue: exactly what row pair 0 (rows 0,1; pre rows 0..17)
            # needs, plus one tile of slack on each tensor
            for w in ([("cur", 0)] + [("pre", i) for i in range(5)]
                      + [("cur", 1), ("pre", 5)]):
                do_chunk(w)
            # remaining stage-A chunks, ordered so that the tiles needed by
            # prework(t+2) (rows 2t+4, 2t+5) exist one pair early
            due = {}
            hp, hc = 6, 2
            npairs_ = HSLICE // 2
            for t in range(npairs_):
                items = []
                need_cur = min((2 * t + 5) // CUR_RPT + 2, CUR_NT)
                need_pre = min((2 * t + 21) // PRE_RPT + 2, PRE_NT)
                while hc < need_cur:
                    items.append(("cur", hc)); hc += 1
                while hp < need_pre:
                    items.append(("pre", hp)); hp += 1
                due[t] = items

            _recq = []

            def emit_a(t):
                for w in due.get(t, []):
                    do_chunk_p1(w, t)
                    _recq.append(_p2q[-1][1])
                flush_p2(t)

            def emit_recips():
                while _recq:
                    norm_recips(_recq.pop(0))

            def prework(h):
                """gram + shear + aux DMAs + y build; returns row tiles"""
                stage = stgp.tile([128, STG], bf16, tag="stage", name="stage")
                ct = cur_nb[0][h // CUR_RPT], cur_nb[1][h // CUR_RPT]
                co_ = slice((h % CUR_RPT) * W, (h % CUR_RPT + 1) * W)
                for dy in range(KW):
                    g = gp.tile([128, WP], f32, tag="g", name="g")
                    r = h + dy
                    pt_, po = r // PRE_RPT, (r % PRE_RPT) * WP
                    for c in range(2):
                        nc.tensor.matmul(
                            g[:, :], ct[c][:, co_],
                            pre_nb[c][pt_][:, po:po + WP],
                            start=(c == 0), stop=(c == 1))
                    nc.scalar.activation(stage[:, dy * WP:(dy + 1) * WP],
                                         g[:, :], AF.Copy)
                sc = scr[h % 6]
                wview = sc[0:128 * (STG + 1)].rearrange("(p r) -> p r", r=STG + 1)
                nc.sync.dma_start(wview[:, 0:STG], stage[:, :])
                co = cop.tile([128, M4W], bf16, tag="co", name="co")
                rview = sc[:].rearrange("(p r) -> p r", r=STG + 2)
                rview = rview[:, 0:STG].rearrange("p (a b) -> p a b", b=WP)
                nc.sync.dma_start(co[:, 0:D], rview[:, :, 0:KW])
                axb = cop.tile([128, AUXB], bf16, tag="axb", name="axb")
                nc.sync.dma_start(axb[:, :], auxb_d[h, :, :])
                axi = cop.tile([128, AUXI], i16, tag="axi", name="axi")
                nc.sync.dma_start(axi[:, :], auxi_d[h, :, :])
                y = yp.tile([128, M4W], f32, tag="y", name="y")
                nc.gpsimd.tensor_tensor(out=y[:, :], in0=co[:, :],
                                        in1=axb[:, 0:M4W], op=ALU.add)
                nc.gpsimd.memset(y[:, D:M4W], -9.0)
                cs = csp.tile([128, CS_COLS], f32, tag="cs", name="cs")
                return y, cs, axb, axi

            def compact(y):
                """Compact the 145 live (!= -9.0) values of y[:, 0:289]
                to the left, bit-exactly, via sign/scan/u16-pair scatter.
                Safe because every pixel has exactly 144 tombstones."""
                sg = cpp.tile([128, IW], f32, tag="sg", name="sg")
                nc.scalar.activation(sg[:, :], y[:, :], AF.Sign,
                                     bias=bias85[:, :])
                s = cpp.tile([128, IW], f32, tag="s", name="s")
                nc.vector.tensor_tensor_scan(out=s[:, :], data0=sg[:, :],
                                             data1=sg[:, :], initial=0.0,
                                             op0=ALU.add, op1=ALU.bypass)
                e = cpp.tile([128, IW], f32, tag="e", name="e")
                nc.gpsimd.tensor_scalar(out=e[:, :], in0=sg[:, :],
                                        scalar1=10000.0, scalar2=None,
                                        op0=ALU.mult)
                nc.gpsimd.tensor_tensor(out=e[:, :], in0=e[:, :], in1=s[:, :],
                                        op=ALU.add)
                nc.gpsimd.tensor_tensor(out=e[:, :], in0=e[:, :],
                                        in1=iota1[:, :], op=ALU.add)
                i2f = cpp.tile([128, 2 * IW], f32, tag="i2f", name="i2f")
                i2v = i2f[:, :].rearrange("p (n two) -> p n two", two=2)
                nc.gpsimd.tensor_scalar_add(i2v[:, :, 0], e[:, :], 0.0)
                nc.gpsimd.tensor_scalar_add(i2v[:, :, 1], e[:, :], 1.0)
                i2 = cpp.tile([128, 2 * IW], i16, tag="i2", name="i2")
                nc.scalar.activation(i2[:, :], i2f[:, :], AF.Copy)
                zc = zcp.tile([128, ZC_W], f32, tag="zc", name="zc")
                nc.gpsimd.local_scatter(
                    zc[:, :].bitcast(mybir.dt.uint16), y[:, :].bitcast(mybir.dt.uint16),
                    i2[:, :], channels=128, num_elems=2 * ZC_W, num_idxs=2 * IW)
                return zc

            def pmax(out, a, bb, d, r):
                """out = max(a, b) without DVE: d = b - a (Pool),
                r = relu(d) (Act), out = a + r (Pool)."""
                nc.gpsimd.tensor_tensor(out=d[:, :], in0=bb[:, :], in1=a[:, :],
                                        op=ALU.subtract)
                nc.scalar.activation(r[:, :], d[:, :], AF.Relu)
                nc.gpsimd.tensor_tensor(out=out[:, :], in0=a[:, :],
                                        in1=r[:, :], op=ALU.add)

            def postwork(h, y, cs, axb, axi):
                """casts, scatters, combines, transpose, output DMA"""
                cs4b = rcp.tile([128, TOPK], bf16, tag="cs4b", name="cs4b")
                nc.scalar.activation(cs4b[:, :], cs[:, 0:TOPK], AF.Copy,
                                     bias=-4.0)
                csb = rcp.tile([128, NIDXB], bf16, tag="csb", name="csb")
                nc.scalar.activation(csb[:, :], cs[:, 0:NIDXB], AF.Copy)
                tf = rcp.tile([128, TOPK], bf16, tag="tf", name="tf")
                nc.gpsimd.local_scatter(tf[:, :], cs4b[:, :],
                                        axi[:, 0:TOPK], channels=128,
                                        num_elems=TOPK, num_idxs=TOPK)
                yb = rcp.tile([128, TOPK], bf16, tag="yb", name="yb")
                nc.gpsimd.local_scatter(yb[:, :], csb[:, :],
                                        axi[:, TOPK:AUXI], channels=128,
                                        num_elems=TOPK, num_idxs=NIDXB)
                # f = max(cs4b - hm6, min(tf, 0))
                fmain = fbp.tile([128, TOPK], bf16, tag="fm", name="fm")
                nc.gpsimd.tensor_tensor(out=fmain[:, :], in0=cs4b[:, :],
                                        in1=axb[:, M4W:M4W + TOPK],
                                        op=ALU.subtract)
                nc.gpsimd.tensor_scalar_min(tf[:, :], tf[:, :], 0.0)
                ft = fbp.tile([128, TOPK], bf16, tag="ft", name="ft")
                dd = fbp.tile([128, TOPK], f32, tag="dd", name="dd")
                rr = fbp.tile([128, TOPK], f32, tag="rr", name="rr")
                pmax(ft, fmain, tf, dd, rr)
                # b = max(yb - hm10, min(csb[0:256], 0))
                bmain = fbp.tile([128, TOPK], bf16, tag="bm", name="bm")
                nc.gpsimd.tensor_tensor(out=bmain[:, :], in0=yb[:, :],
                                        in1=axb[:, M4W + TOPK:AUXB],
                                        op=ALU.subtract)
                minb = fbp.tile([128, TOPK], bf16, tag="mb", name="mb")
                nc.gpsimd.tensor_scalar_min(minb[:, :], csb[:, 0:TOPK], 0.0)
                bt = fbp.tile([128, TOPK], bf16, tag="bt", name="bt")
                dd2 = fbp.tile([128, TOPK], f32, tag="dd2", name="dd2")
                rr2 = fbp.tile([128, TOPK], f32, tag="rr2", name="rr2")
                pmax(bt, bmain, minb, dd2, rr2)
                for x, out_d in ((ft, f_d), (bt, b_d)):
                    tr = trp.tile([128, TOPK], f32, tag="tr", name="tr")
                    for half in range(2):
                        pt = tp.tile([128, 128], bf16, tag="pt", name="pt")
                        nc.tensor.transpose(
                            pt[:, :], x[:, half * 128:(half + 1) * 128],
                            ident[:, :])
                        nc.scalar.activation(
                            tr[:, half * 128:(half + 1) * 128], pt[:, :],
                            AF.Copy)
                    oview = out_d.rearrange("(cc p) hh ww -> p cc hh ww", cc=2)
                    nc.sync.dma_start(
                        oview[:, :, h, :],
                        tr[:, :].rearrange("p (cc ww) -> p cc ww", ww=W))

            npairs = HSLICE // 2

            def early_round(pr, r):
                (y0, cs0, _, _), (y1, cs1, _, _) = pr
                nc.vector.max(cs0[:, r * 8:(r + 1) * 8], y0[:, 0:D])
                nc.vector.max(cs1[:, r * 8:(r + 1) * 8], y1[:, 0:D])
                nc.vector.match_replace(y0[:, 0:D], cs0[:, r * 8:(r + 1) * 8],
                                        y0[:, 0:D], imm_value=-9.0)
                nc.vector.match_replace(y1[:, 0:D], cs1[:, r * 8:(r + 1) * 8],
                                        y1[:, 0:D], imm_value=-9.0)

            def late_round(zp, r):
                (z0, cs0), (z1, cs1) = zp
                nc.vector.max(cs0[:, r * 8:(r + 1) * 8], z0[:, 0:LIVE])
                nc.vector.max(cs1[:, r * 8:(r + 1) * 8], z1[:, 0:LIVE])
                if r + 1 < NROUND:
                    nc.vector.match_replace(z0[:, 0:LIVE],
                                            cs0[:, r * 8:(r + 1) * 8],
                                            z0[:, 0:LIVE], imm_value=-9.0)
                    nc.vector.match_replace(z1[:, 0:LIVE],
                                            cs1[:, r * 8:(r + 1) * 8],
                                            z1[:, 0:LIVE], imm_value=-9.0)

            # staggered pipeline: block t = early rounds of pair t woven with
            # late rounds of pair t-1; compaction chain emitted mid-block so
            # Pool/Act finish it while DVE drains the remaining late rounds
            pend = {0: (prework(0), prework(1))}
            pend[1] = (prework(2), prework(3))
            zpend = {}
            for t in range(npairs + 1):
                if t + 2 <= npairs - 1:
                    emit_a(t)
                    pend[t + 2] = (prework(2 * t + 4), prework(2 * t + 5))
                early = pend.pop(t) if t < npairs else None
                late = zpend.pop(t - 1) if t > 0 else None
                ei, li = 0, 0
                if early is not None:
                    while ei < NR_EARLY:
                        for _ in range(3):
                            if ei < NR_EARLY:
                                early_round(early, ei)
                                ei += 1
                        if ei == 6:
                            emit_recips()
                        if late is not None and li < NROUND - NR_EARLY:
                            late_round((late[0][:2], late[1][:2]),
                                       NR_EARLY + li)
                            li += 1
                    # a few more late rounds before the compact chain's DVE
                    # scan enters the queue, so Sign (Act) has time to land
                    for _ in range(2):
                        if late is not None and li < NROUND - NR_EARLY:
                            late_round((late[0][:2], late[1][:2]),
                                       NR_EARLY + li)
                            li += 1
                    zpend[t] = (
                        (compact(early[0][0]), early[0][1], early[0][2],
                         early[0][3]),
                        (compact(early[1][0]), early[1][1], early[1][2],
                         early[1][3]))
                if late is not None:
                    emit_recips()
                    while li < NROUND - NR_EARLY:
                        late_round((late[0][:2], late[1][:2]), NR_EARLY + li)
                        li += 1
                    (z0, cs0, axb0, axi0), (z1, cs1, axb1, axi1) = late
                    postwork(2 * (t - 1), z0, cs0, axb0, axi0)
                    postwork(2 * (t - 1) + 1, z1, cs1, axb1, axi1)
            for t in range(npairs - 2, npairs_):
                emit_a(t)
            flush_p2(10**6)
    nc.compile()
    return nc


def _host_prep(pre, curr, mask):
    import ml_dtypes
    bf = ml_dtypes.bfloat16

    pre_pad = np.pad(pre, ((0, 0), (0, 0), (K, K), (K, K)), mode="reflect")
    mask_pad = np.pad(mask, ((0, 0), (0, 0), (K, K), (K, K)))
    ins = []
    jf = np.arange(TOPK, dtype=np.int32)[None, :]
    jb = np.arange(NIDXB, dtype=np.int32)[None, :]
    for k in range(N_CORES):
        b, hh = k // 2, k % 2
        h0 = hh * HSLICE
        mp = mask_pad[b, 0, h0:h0 + HP, :]
        s0, s1 = mp.strides
        m_unf = np.lib.stride_tricks.as_strided(
            mp, (HSLICE, KW, W, KW), (s0, s0, s1, s1))
        m_unf = np.ascontiguousarray(
            m_unf.transpose(0, 2, 1, 3).reshape(HSLICE, W, D))
        kap = m_unf.sum(axis=2).astype(np.int32).reshape(-1, 1)   # [HS*W, 1]
        q = D - kap
        idxf = np.where((jf < kap) & (jf + q < TOPK), jf + q, -1)
        idxb = np.where((jb >= kap) & (jb - kap < TOPK) & (jb <= D - 1),
                        jb - kap, -1)
        auxi = np.concatenate([idxf, idxb], axis=1).astype(np.int16)
        hm6 = np.where(jf >= kap, np.float32(6.0), np.float32(0.0))
        hm10 = np.where(jf >= q, np.float32(10.0), np.float32(0.0))
        m4 = np.zeros((HSLICE * W, M4W), np.float32)
        m4[:, :D] = 4.0 * m_unf.reshape(-1, D)
        auxb = np.concatenate([m4, hm6, hm10], axis=1).astype(bf)
        ins.append({
            "pre_pad": np.ascontiguousarray(pre_pad[b, :, h0:h0 + HP, :]),
            "curr": np.ascontiguousarray(curr[b, :, h0:h0 + HSLICE, :]),
            "auxb": auxb.reshape(HSLICE, W, AUXB),
            "auxi": auxi.reshape(HSLICE, W, AUXI),
        })
    return ins


def kernel(pre, curr, mask, mode):
    from concourse.bass_utils import run_bass_kernel_spmd

    pre = np.asarray(pre, dtype=np.float32)
    curr = np.asarray(curr, dtype=np.float32)
    mask = np.asarray(mask, dtype=np.float32)
    assert int(np.asarray(mode)) == 0

    if _CACHED["nc"] is None:
        _CACHED["nc"] = _build_nc()
    nc = _CACHED["nc"]

    in_maps = _host_prep(pre, curr, mask)
    res = run_bass_kernel_spmd(nc, in_maps, core_ids=list(range(N_CORES)))
    f = np.zeros((B, TOPK, H, W), np.float32)
    bo = np.zeros((B, TOPK, H, W), np.float32)
    for k in range(N_CORES):
        bb, hh = k // 2, k % 2
        f[bb, :, hh * HSLICE:(hh + 1) * HSLICE, :] = res.results[k]["f_out"]
        bo[bb, :, hh * HSLICE:(hh + 1) * HSLICE, :] = res.results[k]["b_out"]
    return (f, bo)
